# revision 9
# baseline (speedup 1.0000x reference)
"""Trainium2 Bass kernel for nn_CustomAttention (dense transformer attention block).

Sharding: 8 cores = 2 batches x 4 head-groups (4 heads each).
Tensor-parallel over heads for q/k/v projections + attention; o_proj computed as
per-core partials (row-sharded contraction) summed on host during unsharding.

Per-core dataflow (all matmuls bf16 operands, fp32 PSUM accumulate):
  XT [h, t] (host-transposed, bf16)
  Q^T/K^T = W^T.T @ XT          -> [d, t] layout, RoPE+bias fused into PSUM evac (DVE)
  V       = XT.T @ WvT          -> [t, d] layout (+ ones column for softmax denom)
  S^T     = K^T.T @ Q^T         -> [kt, qt], exp(scale*S) on ScalarE -> P^T (bf16)
  PV      = P^T.T @ [V | 1]     -> [qt, d+1]; col d = sumexp -> normalize (DVE)
  attn^T  via PE transpose      -> o_proj partial = attn^T.T @ WoT -> [t, o] (f32)
"""
import os
import sys
import numpy as np
import ml_dtypes

sys.path.insert(0, '/opt/trn_rl_repo')

import concourse.bacc as bacc  # noqa: E402
import concourse.tile as tile  # noqa: E402
from concourse import mybir  # noqa: E402
from concourse.bass_utils import run_bass_kernel_spmd  # noqa: E402

F32 = mybir.dt.float32
BF16 = mybir.dt.bfloat16
BF = ml_dtypes.bfloat16

HIDDEN = 2048
HEAD_DIM = 128
S = 2048
B = 2
N_CORES = 8
GROUP_HEADS = 4              # heads per core
GD = GROUP_HEADS * HEAD_DIM  # 512 = local d per core
ROPE_THETA = 10000.0
SCALE = 1.0 / np.sqrt(HEAD_DIM)

NC_H = HIDDEN // 128         # 16 contraction chunks
NT = S // 128                # 16 token tiles
TB = 512                     # token block for projection streaming
NTB = S // TB                # 4

ADD = mybir.AluOpType.add
MULT = mybir.AluOpType.mult
EXP = mybir.ActivationFunctionType.Exp

_PROGRAM = None
_LAST_IN_MAPS = None


def _build_program():
    phases = os.environ.get("K_PHASES", "BCD")
    interleave = os.environ.get("K_INTERLEAVE", "1") == "1"
    nc = bacc.Bacc("TRN2", target_bir_lowering=False)

    xt_d = nc.dram_tensor("xt", [HIDDEN, S], BF16, kind="ExternalInput")
    wq_d = nc.dram_tensor("wq", [HIDDEN, GD], BF16, kind="ExternalInput")
    wk_d = nc.dram_tensor("wk", [HIDDEN, GD], BF16, kind="ExternalInput")
    wv_d = nc.dram_tensor("wv", [HIDDEN, GD], BF16, kind="ExternalInput")
    wo_d = nc.dram_tensor("wo", [GD, HIDDEN], BF16, kind="ExternalInput")
    cos_d = nc.dram_tensor("cos", [128, S], F32, kind="ExternalInput")
    sin_d = nc.dram_tensor("sin", [128, S], F32, kind="ExternalInput")  # sign-folded
    bq_d = nc.dram_tensor("bq", [128, GROUP_HEADS], F32, kind="ExternalInput")
    bk_d = nc.dram_tensor("bk", [128, GROUP_HEADS], F32, kind="ExternalInput")
    bqr_d = nc.dram_tensor("bqr", [128, GROUP_HEADS], F32, kind="ExternalInput")
    bkr_d = nc.dram_tensor("bkr", [128, GROUP_HEADS], F32, kind="ExternalInput")
    idn_d = nc.dram_tensor("idn", [128, 128], BF16, kind="ExternalInput")
    out_d = nc.dram_tensor("out", [S, HIDDEN], F32, kind="ExternalOutput")

    with tile.TileContext(nc) as tc:
        with tc.tile_pool(name="const", bufs=1) as const, \
             tc.tile_pool(name="qk", bufs=1) as qk, \
             tc.tile_pool(name="psb", bufs=4, space="PSUM") as ps_big, \
             tc.tile_pool(name="pss", bufs=4, space="PSUM") as ps_small:

            idn_sb = const.tile([128, 128], BF16)
            nc.sync.dma_start(idn_sb, idn_d[:])
            bq_sb = const.tile([128, GROUP_HEADS], F32)
            bk_sb = const.tile([128, GROUP_HEADS], F32)
            bqr_sb = const.tile([128, GROUP_HEADS], F32)
            bkr_sb = const.tile([128, GROUP_HEADS], F32)
            nc.sync.dma_start(bq_sb, bq_d[:])
            nc.sync.dma_start(bk_sb, bk_d[:])
            nc.sync.dma_start(bqr_sb, bqr_d[:])
            nc.sync.dma_start(bkr_sb, bkr_d[:])
            wo_sb = const.tile([128, GD // 128, HIDDEN], BF16)
            nc.sync.dma_start(wo_sb, wo_d[:].rearrange("(c p) o -> p c o", p=128))

            qT = qk.tile([128, GROUP_HEADS, S], BF16)   # [d, h, t]
            kT = qk.tile([128, GROUP_HEADS, S], BF16)
            v_sb = qk.tile([128, NT, GROUP_HEADS, HEAD_DIM + 1], BF16)
            attn = qk.tile([128, NT, GROUP_HEADS, HEAD_DIM], BF16)
            nc.vector.memset(v_sb[:, :, :, HEAD_DIM:HEAD_DIM + 1], 1.0)

            # ---------------- Phase B: projections + RoPE ----------------
            with tc.tile_pool(name="w", bufs=1) as wpool, \
                 tc.tile_pool(name="cs", bufs=1) as cs, \
                 tc.tile_pool(name="xt", bufs=2) as xtp, \
                 tc.tile_pool(name="tmp", bufs=2) as tmp:
                wq_sb = wpool.tile([128, NC_H, GD], BF16)
                wk_sb = wpool.tile([128, NC_H, GD], BF16)
                wv_sb = wpool.tile([128, NC_H, GD], BF16)
                nc.sync.dma_start(wq_sb, wq_d[:].rearrange("(c p) d -> p c d", p=128))
                nc.sync.dma_start(wk_sb, wk_d[:].rearrange("(c p) d -> p c d", p=128))
                nc.sync.dma_start(wv_sb, wv_d[:].rearrange("(c p) d -> p c d", p=128))
                cos_sb = cs.tile([128, S], F32)
                sin_sb = cs.tile([128, S], F32)
                nc.sync.dma_start(cos_sb, cos_d[:])
                nc.sync.dma_start(sin_sb, sin_d[:])

                for tb in range(NTB):
                    ts_ = slice(tb * TB, (tb + 1) * TB)
                    xt = xtp.tile([128, NC_H, TB], BF16, tag="xt")
                    nc.sync.dma_start(
                        xt, xt_d[:, ts_].rearrange("(c p) t -> p c t", p=128))

                    # Q^T, K^T with fused RoPE evac
                    for wsb, bsb, bsr, dst in (
                            (wq_sb, bq_sb, bqr_sb, qT), (wk_sb, bk_sb, bkr_sb, kT)):
                        for h in range(GROUP_HEADS):
                            ps = ps_big.tile([128, TB], F32, tag="big")
                            for c in range(NC_H):
                                nc.tensor.matmul(
                                    ps, wsb[:, c, h * 128:(h + 1) * 128],
                                    xt[:, c, :],
                                    start=(c == 0), stop=(c == NC_H - 1))
                            # q' = (q+b)*cos + rot_half(q+b)*sin_signed
                            t1 = tmp.tile([128, TB], F32, tag="t1")
                            t2 = tmp.tile([128, TB], F32, tag="t2")
                            nc.vector.scalar_tensor_tensor(
                                t1, ps, bsb[:, h:h + 1], cos_sb[:, ts_],
                                op0=ADD, op1=MULT)
                            nc.vector.scalar_tensor_tensor(
                                t2[0:64], ps[64:128], bsr[0:64, h:h + 1],
                                sin_sb[0:64, ts_], op0=ADD, op1=MULT)
                            nc.vector.scalar_tensor_tensor(
                                t2[64:128], ps[0:64], bsr[64:128, h:h + 1],
                                sin_sb[64:128, ts_], op0=ADD, op1=MULT)
                            nc.vector.tensor_add(dst[:, h, ts_], t1, t2)

                    # V (token-partition layout), cast to bf16
                    for tt in range(TB // 128):
                        ktile = tb * (TB // 128) + tt
                        ps = ps_big.tile([128, GD], F32, tag="big")
                        for c in range(NC_H):
                            nc.tensor.matmul(
                                ps, xt[:, c, tt * 128:(tt + 1) * 128],
                                wv_sb[:, c, :],
                                start=(c == 0), stop=(c == NC_H - 1))
                        nc.scalar.copy(
                            v_sb[:, ktile, :, 0:HEAD_DIM],
                            ps[:].rearrange("p (a b) -> p a b", a=GROUP_HEADS))

            # ---------------- Phase C: attention ----------------
            if "C" in phases:
                groups = [(h, qg) for h in range(GROUP_HEADS) for qg in range(NTB)]
                with tc.tile_pool(name="pt", bufs=2) as ptp, \
                     tc.tile_pool(name="rec", bufs=4) as recp:
                    pt_tiles = {}

                    def emit_st(i, kt):
                        h, qg = groups[i]
                        if kt == 0:
                            pt_tiles[i] = ptp.tile(
                                [128, NT, TB], BF16, tag="pt", name=f"pt{i}")
                        ps = ps_big.tile([128, TB], F32, tag="big")
                        nc.tensor.matmul(
                            ps, kT[:, h, kt * 128:(kt + 1) * 128],
                            qT[:, h, qg * TB:(qg + 1) * TB],
                            start=True, stop=True)
                        nc.scalar.activation(
                            pt_tiles[i][:, kt, :], ps, EXP, scale=float(SCALE))

                    def emit_pv_group(i, s_):
                        # one contiguous 16-MM PSUM accumulation + its evac
                        h, qg = groups[i]
                        pt = pt_tiles[i]
                        pv = ps_small.tile([128, HEAD_DIM + 1], F32,
                                           tag="small", name=f"pv{i}_{s_}")
                        for kt in range(NT):
                            nc.tensor.matmul(
                                pv, pt[:, kt, s_ * 128:(s_ + 1) * 128],
                                v_sb[:, kt, h, :],
                                start=(kt == 0), stop=(kt == NT - 1))
                        qtile = qg * 4 + s_
                        rec = recp.tile([128, 1], F32, tag="rec",
                                        name=f"rec{i}_{s_}")
                        nc.vector.reciprocal(rec, pv[:, HEAD_DIM:HEAD_DIM + 1])
                        nc.vector.tensor_scalar_mul(
                            attn[:, qtile, h, :], pv[:, 0:HEAD_DIM], rec)

                    if interleave:
                        for kt in range(NT):
                            emit_st(0, kt)
                        for i in range(len(groups)):
                            for s_ in range(4):
                                if i + 1 < len(groups):
                                    for kt in range(4 * s_, 4 * s_ + 4):
                                        emit_st(i + 1, kt)
                                emit_pv_group(i, s_)
                            del pt_tiles[i]
                    else:
                        for i in range(len(groups)):
                            for kt in range(NT):
                                emit_st(i, kt)
                            for s_ in range(4):
                                emit_pv_group(i, s_)
                            del pt_tiles[i]

            # ---------------- Phase D: transpose + o_proj ----------------
            if "D" in phases and "C" in phases:
                with tc.tile_pool(name="att", bufs=2) as attp, \
                     tc.tile_pool(name="osb", bufs=2) as outp:
                    for ttile in range(NT):
                        at_t = attp.tile([128, GROUP_HEADS, 128], BF16, tag="atT")
                        for h in range(GROUP_HEADS):
                            trp = ps_small.tile([128, 128], BF16, tag="small")
                            nc.tensor.transpose(trp, attn[:, ttile, h, :], idn_sb)
                            nc.scalar.copy(at_t[:, h, :], trp)
                        os_ = outp.tile([128, HIDDEN], F32, tag="osb")
                        for og in range(4):
                            ps = ps_big.tile([128, 512], F32, tag="big")
                            for dc in range(GROUP_HEADS):
                                nc.tensor.matmul(
                                    ps, at_t[:, dc, :],
                                    wo_sb[:, dc, og * 512:(og + 1) * 512],
                                    start=(dc == 0), stop=(dc == GROUP_HEADS - 1))
                            nc.scalar.copy(os_[:, og * 512:(og + 1) * 512], ps)
                        nc.sync.dma_start(
                            out_d[ttile * 128:(ttile + 1) * 128, :], os_)
            else:
                # debug dump path: write intermediates so 'out' is produced
                with tc.tile_pool(name="osb0", bufs=2) as outp0:
                    for ttile in range(NT):
                        os_ = outp0.tile([128, HIDDEN], F32, tag="osb0")
                        if "C" in phases:
                            nc.vector.tensor_copy(
                                os_[:, 0:GROUP_HEADS * HEAD_DIM],
                                attn[:, ttile, :, :].rearrange(
                                    "p a b -> p (a b)"))
                        else:
                            nc.vector.tensor_copy(
                                os_[:, 0:GROUP_HEADS * (HEAD_DIM + 1)],
                                v_sb[:, ttile, :, :].rearrange(
                                    "p a b -> p (a b)"))
                            nc.vector.tensor_copy(
                                os_[:, 520:520 + 512], qT[:, ttile % 4, 0:512])
                            nc.vector.tensor_copy(
                                os_[:, 1088:1088 + 512], kT[:, ttile % 4, 0:512])
                        nc.sync.dma_start(
                            out_d[ttile * 128:(ttile + 1) * 128, :], os_)

    nc.compile()
    return nc


def _get_program():
    global _PROGRAM
    if _PROGRAM is None:
        _PROGRAM = _build_program()
    return _PROGRAM


def _rope_tables(position_ids_b):
    pos = np.asarray(position_ids_b).astype(np.float32)  # [S]
    inv_freq = (1.0 / (ROPE_THETA ** (
        np.arange(0, HEAD_DIM, 2, dtype=np.float32) / HEAD_DIM))).astype(np.float32)
    freqs = np.outer(pos, inv_freq)                      # [S, 64]
    emb = np.concatenate([freqs, freqs], axis=-1)        # [S, 128]
    cos_t = np.ascontiguousarray(np.cos(emb).T)          # [128, S] f32
    sin_t = np.ascontiguousarray(np.sin(emb).T)
    sin_signed = sin_t.copy()
    sin_signed[0:64] = -sin_t[0:64]
    return cos_t, sin_signed


def kernel(hidden_states, position_ids, Wq, bq, Wk, bk, Wv, bv, Wo):
    hidden_states = np.asarray(hidden_states, dtype=np.float32)
    position_ids = np.asarray(position_ids)
    Wq, Wk, Wv, Wo = (np.asarray(w, dtype=np.float32) for w in (Wq, Wk, Wv, Wo))
    bq, bk, bv = (np.asarray(x, dtype=np.float32) for x in (bq, bk, bv))

    nc = _get_program()
    idn = np.eye(128, dtype=BF)

    xt_b = [np.ascontiguousarray(hidden_states[b].T).astype(BF) for b in range(B)]
    tables = [_rope_tables(position_ids[b]) for b in range(B)]

    in_maps = []
    for core in range(N_CORES):
        b, g = core // 4, core % 4
        sl = slice(g * GD, (g + 1) * GD)
        cos_t, sin_s = tables[b]
        in_maps.append({
            "xt": xt_b[b],
            "wq": np.ascontiguousarray(Wq[sl, :].T).astype(BF),
            "wk": np.ascontiguousarray(Wk[sl, :].T).astype(BF),
            "wv": np.ascontiguousarray(Wv[sl, :].T).astype(BF),
            "wo": np.ascontiguousarray(Wo[:, sl].T).astype(BF),
            "cos": cos_t,
            "sin": sin_s,
            "bq": np.ascontiguousarray(bq[sl].reshape(GROUP_HEADS, 128).T),
            "bk": np.ascontiguousarray(bk[sl].reshape(GROUP_HEADS, 128).T),
            "bqr": np.ascontiguousarray(
                np.roll(bq[sl].reshape(GROUP_HEADS, 128).T, 64, axis=0)),
            "bkr": np.ascontiguousarray(
                np.roll(bk[sl].reshape(GROUP_HEADS, 128).T, 64, axis=0)),
            "idn": idn,
        })

    global _LAST_IN_MAPS
    _LAST_IN_MAPS = in_maps
    res = run_bass_kernel_spmd(nc, in_maps, list(range(N_CORES)))

    out = np.zeros((B, S, HIDDEN), dtype=np.float32)
    for core in range(N_CORES):
        out[core // 4] += res.results[core]["out"]
    # v-bias passes through softmax averaging exactly: out += Wo @ bv
    if np.any(bv):
        out += (Wo @ bv)[None, None, :]
    return out
